# revision 1
# baseline (speedup 1.0000x reference)
"""Mean-field CRF message passing on 8 Trainium2 NeuronCores.

Math: the reference builds PP[b] = gaussian * (1 - sim) * W_sym (N x N per
batch) and iterates l <- unary + PP @ (2*sigmoid(l) - 1) ten times.  PP is
rank-structured:

    PP[n,m] = g_n * g_m * (1 - u_n . u_m) * W_sym[n,m]
    with g = exp(-|f|^2/2), u = f/|f|  (per batch)

so PP @ m = g*(W_sym v0) - (g*u0)*(W_sym v1) - (g*u1)*(W_sym v2) with
v0 = g*m, v1 = u0*v0, v2 = u1*v0 — PP is never materialized.  Per iteration
this is one (N x N) @ (N x 12) matmul shared across the 4 batches.

Distribution: W_sym rows are sharded 512/core (4 MB bf16, SBUF-resident).
Each iteration every core computes y for its own rows (contracting over all
N with V built locally from the gathered message vector m), applies the
elementwise tail to get its slice of the new m, and an 8 KB AllGather shares
m for the next iteration.  The matmul is 4x column-tiled (tile_position) so
four k-tiles stream through the PE concurrently.
"""

import sys

sys.path.insert(0, "/opt/trn_rl_repo")

import numpy as np
import ml_dtypes

import concourse.bacc as bacc
import concourse.mybir as mybir
import concourse.tile as tile
from concourse.bass_utils import run_bass_kernel_spmd

N = 4096
B = 4
ITERS = 10
CORES = 8
R = N // CORES            # 512 rows per core
KT = N // 128             # 32 k-tiles of 128
TL = R // 128             # 4 row-tiles of 128 per core
C = 12                    # channels: c = 4*vec + b, vec in {0,1,2}
F32 = mybir.dt.float32
BF16 = mybir.dt.bfloat16

_NC_CACHE = {}


def _build():
    nc = bacc.Bacc("TRN2", target_bir_lowering=False, debug=False, num_devices=CORES)

    unary_d = nc.dram_tensor("unary", [128, TL * B], F32, kind="ExternalInput")
    hown_d = nc.dram_tensor("hown", [128, TL * C], F32, kind="ExternalInput")
    gf_d = nc.dram_tensor("gf", [128, KT * B], F32, kind="ExternalInput")
    u01f_d = nc.dram_tensor("u01f", [128, KT * 2 * B], F32, kind="ExternalInput")
    sel_d = nc.dram_tensor("sel", [96 + C, C], F32, kind="ExternalInput")
    w_d = nc.dram_tensor("w", [KT, 128, R], BF16, kind="ExternalInput")
    out_d = nc.dram_tensor("out", [128, TL * B], F32, kind="ExternalOutput")

    with tile.TileContext(nc) as tc:
        with (
            tc.tile_pool(name="persist", bufs=1) as persist,
            tc.tile_pool(name="work", bufs=2) as work,
            tc.tile_pool(name="psum", bufs=2, space="PSUM") as psum,
            tc.tile_pool(name="dram", bufs=2, space="DRAM") as dram,
        ):
            # --- persistent SBUF state ---
            unary = persist.tile([128, TL * B], F32)
            hown = persist.tile([128, TL * C], F32)
            gf = persist.tile([128, KT * B], F32)
            u01f = persist.tile([128, KT * 2 * B], F32)
            sel = persist.tile([96 + C, C], F32)
            W_sb = persist.tile([128, KT, R], BF16)       # 4 MB weight shard
            Vfull = persist.tile([128, KT, C], BF16)      # V for all rows (96 KB)
            mfull = persist.tile([128, KT * B], F32)      # gathered m (64 KB)

            # Small inputs first: iteration 0 only needs `unary` to reach the
            # first AllGather trigger; W streams in behind it.
            nc.sync.dma_start(unary[:], unary_d[:])
            nc.sync.dma_start(hown[:], hown_d[:])
            nc.sync.dma_start(gf[:], gf_d[:])
            nc.sync.dma_start(u01f[:], u01f_d[:])
            nc.sync.dma_start(sel[:], sel_d[:])
            for ch in range(8):
                t0, t1 = 4 * ch, 4 * (ch + 1)
                nc.sync.dma_start(
                    W_sb[:, t0:t1, :],
                    w_d[t0:t1, :, :].rearrange("t p j -> p t j"),
                )

            hown3 = hown[:].rearrange("p (t c) -> p t c", t=TL)
            gf3 = gf[:].rearrange("p (t b) -> p t b", t=KT)
            u01f4 = u01f[:].rearrange("p (t d b) -> p t d b", t=KT, d=2)

            l_cur = unary
            for it in range(ITERS):
                # m_own = 2*sigmoid(l) - 1 == tanh(l/2): one ScalarE op.
                mown = work.tile([128, TL * B], F32, name="mown")
                nc.scalar.activation(
                    mown[:], l_cur[:], mybir.ActivationFunctionType.Tanh,
                    scale=0.5,
                )

                # AllGather m (8 KB per core).
                vin = dram.tile([128, TL * B], F32, name="vin")
                vout = dram.tile([CORES, 128, TL * B], F32, name="vout")
                nc.sync.dma_start(vin[:], mown[:])
                nc.gpsimd.collective_compute(
                    "AllGather",
                    mybir.AluOpType.bypass,
                    replica_groups=[list(range(CORES))],
                    ins=[vin.opt()],
                    outs=[vout.opt()],
                )
                # Two engines issue the gather-to-SBUF halves in parallel
                # (the pattern is descriptor-bound: 64 B per partition/rank).
                mfullr = mfull[:].rearrange("p (r f) -> p r f", r=CORES)
                voutr = vout[:].rearrange("r p f -> p r f")
                half = CORES // 2
                nc.sync.dma_start(mfullr[:, 0:half, :], voutr[:, 0:half, :])
                nc.scalar.dma_start(mfullr[:, half:, :], voutr[:, half:, :])

                # V = [g*m, u0*g*m, u1*g*m] for all rows.
                mfull3 = mfull[:].rearrange("p (t b) -> p t b", t=KT)
                nc.vector.tensor_mul(Vfull[:, :, 0:B], mfull3, gf3)
                nc.vector.tensor_mul(
                    Vfull[:, :, B:3 * B].rearrange("p t (d b) -> p t d b", d=2),
                    Vfull[:, :, 0:B].unsqueeze(2).broadcast_to([128, KT, 2, B]),
                    u01f4,
                )

                # yT[c, j] = sum_row V[row, c] * W_sym[row, own_col j]
                # 4x column-tiled: strip j of PSUM accumulates k-tiles 4r+j.
                yT_ps = psum.tile([128, R], F32, name="yT_ps")
                for r in range(CORES):
                    for j in range(4):
                        t = 4 * r + j
                        nc.tensor.matmul(
                            yT_ps[32 * j:32 * j + C, :],
                            Vfull[:, t, :],
                            W_sb[:, t, :],
                            start=(r == 0),
                            stop=(r == CORES - 1),
                            tile_position=(0, 32 * j),
                        )
                # Fused strip-sum + transpose: one PSUM->SBUF copy of all
                # strips (junk partitions included), then per row-tile one
                # matmul against a stacked-identity selector:
                # yB[p, c] = sum_k yT_all[k, p] * sel[k, c], sel zero on junk.
                yT_sb = work.tile([96 + C, R], F32, name="yT_sb")
                nc.vector.tensor_copy(yT_sb[:], yT_ps[0:96 + C, :])
                yB_ps = psum.tile([128, TL * C], F32, name="yB_ps")
                yB3 = yB_ps[:].rearrange("p (t c) -> p t c", t=TL)
                for tl in range(TL):
                    nc.tensor.matmul(
                        yB3[:, tl, :],
                        yT_sb[:, 128 * tl:128 * (tl + 1)],
                        sel[:],
                        start=True, stop=True,
                    )

                # E = g*y0 - (g*u0)*y1 - (g*u1)*y2 ; l = unary + E
                p_ = work.tile([128, TL * C], F32, name="p_")
                nc.vector.tensor_mul(p_[:], yB_ps[:], hown[:])
                p3 = p_[:].rearrange("p (t c) -> p t c", t=TL)
                e_ = work.tile([128, TL * B], F32, name="e_")
                e3 = e_[:].rearrange("p (t b) -> p t b", t=TL)
                nc.vector.tensor_sub(e3, p3[:, :, 0:B], p3[:, :, B:2 * B])
                nc.vector.tensor_sub(e3, e3, p3[:, :, 2 * B:3 * B])
                l_nxt = work.tile([128, TL * B], F32, name="l_nxt")
                nc.vector.tensor_add(l_nxt[:], unary[:], e_[:])
                l_cur = l_nxt

            nc.sync.dma_start(out_d[:], l_cur[:])

    nc.compile()
    return nc


def _host_prep(delta_p, logits, W):
    feats = np.asarray(delta_p, dtype=np.float32).reshape(B, N, 2)
    r2 = feats[..., 0] ** 2 + feats[..., 1] ** 2
    nrm = np.sqrt(r2)
    g = np.exp(-r2 / 2.0)                      # (B, N)
    u0 = feats[..., 0] / nrm
    u1 = feats[..., 1] / nrm
    Wf = np.asarray(W, dtype=np.float32)[0]
    Wsym = (Wf + Wf.T) * 0.5                   # (N, N)
    unary = np.asarray(logits, dtype=np.float32)[:, :, 0]  # (B, N)

    def own_layout(X, k):
        # (..., B, N) -> (128, TL, ..., B) for this core's rows
        blk = X[..., R * k:R * (k + 1)]                  # (..., B, 512)
        order = np.moveaxis(blk, -1, 0)                  # (512, ..., B)
        s = order.shape
        return np.ascontiguousarray(
            order.reshape(TL, 128, *s[1:]).transpose(1, 0, *range(2, 2 + len(s) - 1))
        ).reshape(128, -1)

    def full_layout(X):
        # (..., B, N) -> (128, KT, ..., B)
        order = np.moveaxis(X, -1, 0)                    # (N, ..., B)
        s = order.shape
        return np.ascontiguousarray(
            order.reshape(KT, 128, *s[1:]).transpose(1, 0, *range(2, 2 + len(s) - 1))
        ).reshape(128, -1)

    h = np.stack([g, g * u0, g * u1])                    # (3, B, N)
    u01 = np.stack([u0, u1])                             # (2, B, N)
    gf = full_layout(g)
    u01f = full_layout(u01)
    sel = np.zeros((96 + C, C), dtype=np.float32)
    for j in range(4):
        sel[32 * j:32 * j + C] = np.eye(C, dtype=np.float32)

    in_maps = []
    for k in range(CORES):
        wk = np.ascontiguousarray(
            Wsym[:, R * k:R * (k + 1)].reshape(KT, 128, R)
        ).astype(ml_dtypes.bfloat16)
        in_maps.append({
            "unary": own_layout(unary, k),
            "hown": own_layout(h, k),
            "gf": gf,
            "u01f": u01f,
            "sel": sel,
            "w": wk,
        })
    return in_maps


def _assemble(results):
    outs = np.stack([results[k]["out"] for k in range(CORES)])  # (8, 128, TL*B)
    outs = outs.reshape(CORES, 128, TL, B)
    l = outs.transpose(3, 0, 2, 1).reshape(B, N)               # [b, 512k+128tl+p]
    return np.ascontiguousarray(l)[:, :, None].astype(np.float32)


def kernel(delta_p, logits, W):
    if "nc" not in _NC_CACHE:
        _NC_CACHE["nc"] = _build()
    nc = _NC_CACHE["nc"]
    in_maps = _host_prep(delta_p, logits, W)
    res = run_bass_kernel_spmd(nc, in_maps, core_ids=list(range(CORES)))
    return _assemble(res.results)



# revision 6
# speedup vs baseline: 1.1834x; 1.1834x over previous
"""Mean-field CRF message passing on 8 Trainium2 NeuronCores.

Math: the reference builds PP[b] = gaussian * (1 - sim) * W_sym (N x N per
batch) and iterates l <- unary + PP @ (2*sigmoid(l) - 1) ten times.  PP is
rank-structured:

    PP[n,m] = g_n * g_m * (1 - u_n . u_m) * W_sym[n,m]
    with g = exp(-|f|^2/2), u = f/|f|  (per batch)

so PP @ m needs only y_v = W_sym^T (h_v * m), v=0..2, h = [g, g*u0, g*u1],
then E = sum_v sign_v * h_v * y_v — PP is never materialized.  Per iteration
this is one (N x N) @ (N x 12) matmul shared across the 4 batches.

Distribution: W_sym columns are sharded 512/core (4 MB bf16, SBUF-resident,
loaded with a per-partition-contiguous DMA).  Each iteration every core
computes y for its own columns, applies the fused elementwise tail
(h-factors and signs folded into one PSUM-read multiply + a 0/1 selector
matmul that both transposes and channel-reduces), and an AllGather shares
the bf16 message vector m for the next iteration.

Index bookkeeping (all permutations host-side): global row/col
n = 512k + c with own-col index c = 32*p2 + t = 4*p + tau.  As a
contraction row, n lives at SBUF partition P = 16k + p2, k-tile T = t; as
core k's own output column it lives at l-layout partition p = c//4,
transpose-block tau = c%4 (W column order j = 128*tau + p).  With this
mapping mown's flat [p][(tau b)] order equals the rank-block order the
receivers need, so the bounce-out is a dense copy and the AllGather output
lands in SBUF with ONE per-partition-contiguous DMA (vout -> mfull),
instead of a 64B-granular scatter.
"""

import sys

sys.path.insert(0, "/opt/trn_rl_repo")

import numpy as np
import ml_dtypes

import concourse.bacc as bacc
import concourse.mybir as mybir
import concourse.tile as tile
from concourse.bass_utils import run_bass_kernel_spmd

N = 4096
B = 4
ITERS = 10
CORES = 8
R = N // CORES            # 512 own columns per core
KT = N // 128             # 32 k-tiles of 128
TL = R // 128             # 4 transpose blocks (tau) per core
C = 12                    # channels: c = 4*v + b, v in {0,1,2}
F32 = mybir.dt.float32
BF16 = mybir.dt.bfloat16

_NC_CACHE = {}


def _build():
    nc = bacc.Bacc("TRN2", target_bir_lowering=False, debug=False, num_devices=CORES)

    unary_d = nc.dram_tensor("unary", [128, TL * B], F32, kind="ExternalInput")
    hfac_d = nc.dram_tensor("hfac", [96 + C, R], F32, kind="ExternalInput")
    hgf_d = nc.dram_tensor("hgf", [128, KT * 3 * B], F32, kind="ExternalInput")
    sel_d = nc.dram_tensor("sel", [96 + C, B], BF16, kind="ExternalInput")
    w_d = nc.dram_tensor("w", [128, KT * R], BF16, kind="ExternalInput")
    out_d = nc.dram_tensor("out", [128, TL * B], F32, kind="ExternalOutput")

    with tile.TileContext(nc) as tc:
        with (
            tc.tile_pool(name="persist", bufs=1) as persist,
            tc.tile_pool(name="work", bufs=2) as work,
            tc.tile_pool(name="psum", bufs=2, space="PSUM") as psum,
            tc.tile_pool(name="dram", bufs=2, space="DRAM") as dram,
        ):
            # --- persistent SBUF state ---
            unary = persist.tile([128, TL * B], F32)
            hfac = persist.tile([96 + C, R], F32)
            hgf = persist.tile([128, KT * 3 * B], F32)
            sel = persist.tile([96 + C, B], BF16)
            W_sb = persist.tile([128, KT, R], BF16)       # 4 MB weight shard
            Vfull = persist.tile([128, KT, C], BF16)      # V for all rows (96 KB)
            mfull = persist.tile([128, KT * B], BF16)     # gathered m (32 KB)

            # Small inputs first: iteration 0 only needs `unary` to reach the
            # first AllGather trigger; W streams in behind it with a single
            # per-partition-contiguous 4 MB DMA.
            nc.sync.dma_start(unary[:], unary_d[:])
            nc.sync.dma_start(hfac[:], hfac_d[:])
            nc.sync.dma_start(hgf[:], hgf_d[:])
            nc.sync.dma_start(sel[:], sel_d[:])
            Wv = W_sb[:].rearrange("p t j -> p (t j)")
            for h in range(4):
                lo, hi = h * (KT * R // 4), (h + 1) * (KT * R // 4)
                nc.sync.dma_start(Wv[:, lo:hi], w_d[:, lo:hi])

            hgf4 = hgf[:].rearrange("p (t v b) -> p t v b", t=KT, v=3)
            mfull3 = mfull[:].rearrange("p (t b) -> p t b", t=KT)
            Vfull4 = Vfull[:].rearrange("p t (v b) -> p t v b", v=3)

            l_cur = unary
            for it in range(ITERS):
                # m_own = 2*sigmoid(l) - 1 == tanh(l/2): one ScalarE op,
                # bf16 output (exchange runs in bf16).
                mown = work.tile([128, TL * B], BF16, name="mown")
                nc.scalar.activation(
                    mown[:], l_cur[:], mybir.ActivationFunctionType.Tanh,
                    scale=0.5,
                )

                # AllGather m (4 KB per core).  With own-col c = 4p + tau,
                # mown's flat [p][(tau b)] order IS the rank-block layout the
                # receivers need — the bounce-out is a verbatim dense copy.
                vin = dram.tile([128, TL * B], BF16, name="vin")
                vout = dram.tile([128, KT * B], BF16, name="vout")
                nc.sync.dma_start(vin[:], mown[:])
                nc.gpsimd.collective_compute(
                    "AllGather",
                    mybir.AluOpType.bypass,
                    replica_groups=[list(range(CORES))],
                    ins=[vin.opt()],
                    outs=[vout.opt()],
                )
                # One per-partition-contiguous 256 B/partition load.
                nc.sync.dma_start(mfull[:], vout[:])

                # V[:, T, (v,b)] = h_v * m for all rows: single DVE op.
                nc.vector.tensor_mul(
                    Vfull4,
                    hgf4,
                    mfull3.unsqueeze(2).broadcast_to([128, KT, 3, B]),
                )

                # yT[c, j] = sum_row V[row, c] * W_sym[row, own_col j]
                # 4x column-tiled: strip j of PSUM accumulates k-tiles 4r+j.
                yT_ps = psum.tile([128, R], F32, name="yT_ps")
                for r in range(CORES):
                    for j in range(4):
                        t = 4 * r + j
                        nc.tensor.matmul(
                            yT_ps[32 * j:32 * j + C, :],
                            Vfull[:, t, :],
                            W_sb[:, t, :],
                            start=(r == 0),
                            stop=(r == CORES - 1),
                            tile_position=(0, 32 * j),
                        )
                # Fused tail: P = yT * (sign_v * h_v[col]) straight out of
                # PSUM (junk strip rows zeroed via hfac), then per tau-block
                # one matmul against the 0/1 selector both transposes and
                # reduces channels: E[p, (tau, b)] = sum_k P[k, 128tau+p] sel[k, b].
                P_ = work.tile([96 + C, R], BF16, name="P_")
                nc.vector.tensor_mul(P_[:], yT_ps[0:96 + C, :], hfac[:])
                yB_ps = psum.tile([128, TL * B], F32, name="yB_ps")
                yB3 = yB_ps[:].rearrange("p (t b) -> p t b", t=TL)
                for tl in range(TL):
                    nc.tensor.matmul(
                        yB3[:, tl, :],
                        P_[:, 128 * tl:128 * (tl + 1)],
                        sel[:],
                        start=True, stop=True,
                    )

                # l = unary + E
                l_nxt = work.tile([128, TL * B], F32, name="l_nxt")
                nc.vector.tensor_add(l_nxt[:], unary[:], yB_ps[:])
                l_cur = l_nxt

            nc.sync.dma_start(out_d[:], l_cur[:])

    nc.compile()
    return nc


def _perms():
    """Index maps of the layout described in the module docstring."""
    # contraction rows: flat (P, T) -> global n
    P = np.arange(128)
    T = np.arange(KT)
    rowperm = (512 * (P[:, None] // 16) + 32 * (P[:, None] % 16) + T[None, :])
    # own columns: W column j (= 128*tau + p) -> own col index c = 4p + tau
    j = np.arange(R)
    colperm = 4 * (j % 128) + j // 128
    # l-layout: (partition p, tau) -> own col index c = 4p + tau
    c_l = 4 * np.arange(128)[:, None] + np.arange(TL)[None, :]
    return rowperm, colperm, c_l


def _host_prep(delta_p, logits, W):
    feats = np.asarray(delta_p, dtype=np.float32).reshape(B, N, 2)
    r2 = feats[..., 0] ** 2 + feats[..., 1] ** 2
    nrm = np.sqrt(r2)
    g = np.exp(-r2 / 2.0)                      # (B, N)
    u0 = feats[..., 0] / nrm
    u1 = feats[..., 1] / nrm
    h = np.stack([g, g * u0, g * u1])          # (3, B, N)
    sign = np.array([1.0, -1.0, -1.0], dtype=np.float32)
    Wf = np.asarray(W, dtype=np.float32)[0]
    Wsym = (Wf + Wf.T) * 0.5                   # (N, N)
    unary = np.asarray(logits, dtype=np.float32)[:, :, 0]  # (B, N)

    rowperm, colperm, c_l = _perms()
    # rows permuted once for all cores: (128*KT, N) -> (128, KT, N)
    Wrows = Wsym[rowperm.reshape(-1)].reshape(128, KT, N)

    # full-layout h for the V build: hgf[P, (T, v, b)] = h_v[b, n(P,T)]
    hgf = np.ascontiguousarray(
        h[:, :, rowperm].transpose(2, 3, 0, 1)          # (128, KT, 3, B)
    ).reshape(128, KT * 3 * B)

    sel = np.zeros((96 + C, B), dtype=np.float32)
    for s in range(4):
        for v in range(3):
            sel[32 * s + 4 * v:32 * s + 4 * v + B] = np.eye(B, dtype=np.float32)
    sel = sel.astype(ml_dtypes.bfloat16)

    in_maps = []
    for k in range(CORES):
        cols = R * k + colperm                          # global own cols, j-order
        wk = np.ascontiguousarray(Wrows[:, :, cols]).astype(ml_dtypes.bfloat16)
        # hfac[(32s + 4v + b), j] = sign_v * h_v[b, own col j]; junk rows 0
        hf = np.zeros((96 + C, R), dtype=np.float32)
        for s in range(4):
            for v in range(3):
                hf[32 * s + 4 * v:32 * s + 4 * v + B] = sign[v] * h[v][:, cols]
        in_maps.append({
            "unary": np.ascontiguousarray(unary[:, R * k + c_l].transpose(1, 2, 0)
                                          ).reshape(128, TL * B),
            "hfac": hf,
            "hgf": hgf,
            "sel": sel,
            "w": wk.reshape(128, KT * R),
        })
    return in_maps


def _assemble(results):
    _, _, c_l = _perms()
    l = np.empty((B, N), dtype=np.float32)
    for k in range(CORES):
        blk = results[k]["out"].reshape(128, TL, B)     # (p, tau, b)
        l[:, R * k + c_l] = blk.transpose(2, 0, 1)      # (B, p, tau)
    return np.ascontiguousarray(l)[:, :, None].astype(np.float32)


def kernel(delta_p, logits, W):
    if "nc" not in _NC_CACHE:
        _NC_CACHE["nc"] = _build()
    nc = _NC_CACHE["nc"]
    in_maps = _host_prep(delta_p, logits, W)
    res = run_bass_kernel_spmd(nc, in_maps, core_ids=list(range(CORES)))
    return _assemble(res.results)
